# revision 13
# baseline (speedup 1.0000x reference)
"""Trainium2 Bass kernel for nn_ActorCritic (CNN + MLP + 19-node GNN, batch 1).

Strategy: the model is tiny except fc0_w (512x7200 f32 = 14.7MB), and the
network is a single serial chain ending in 20 output scalars, so there is no
way to split it across cores without a cross-core combine -- and on this
runtime every cross-core primitive (collective_compute, remote DMA) costs
~85us in entry-barrier/firmware latency, dwarfing the 5us saved on DMA.
So each of the 8 cores runs the full network independently (identical
outputs; core 0's is returned).  The dominant fc0_w stream is cast to
bfloat16 on the host (rel-err ~3e-3, well inside tolerance), halving the
memory-bound phase, and is consumed by the TensorEngine as 512-wide moving
operands while the conv chain overlaps the stream.
"""

import numpy as np
import ml_dtypes

import concourse.bacc as bacc
import concourse.mybir as mybir
from concourse.tile import TileContext
from concourse.bass_utils import run_bass_kernel_spmd

N_CORES = 8
F32 = mybir.dt.float32
BF16 = mybir.dt.bfloat16
AL = mybir.AluOpType
AF = mybir.ActivationFunctionType

NODE = 19
BN_EPS = 1e-5

# fc0 k-dim padded layout: k' = c*256 + j   (c<32 conv2-channels, j<225 pixels)
KPAD = 32 * 256          # 8192
NCHUNK = KPAD // 128     # 64


# --------------------------------------------------------------------------
# host-side input prep (pure relayouts / gathers, no model arithmetic)
# --------------------------------------------------------------------------

def _host_prep(inputs):
    d = {}
    x = np.asarray(inputs["x"], np.float32).reshape(125, 125)

    # conv1 im2col, grouped by maxpool 2x2 output parity.
    # conv1: 5x5 stride 2 pad 1 -> 62x62; pool 2x2 -> 31x31 per parity group.
    xp = np.zeros((128, 128), np.float32)
    xp[1:126, 1:126] = x  # zero pad=1 (plus dead rows/cols beyond)
    col = np.empty((25, 4, 961), np.float32)
    for ky in range(5):
        for kx in range(5):
            # conv out (y,x): input (2y+ky, 2x+kx) in padded coords
            patch = xp[ky:ky + 124:2, kx:kx + 124:2]          # [62, 62]
            g = 0
            for py in range(2):
                for px in range(2):
                    col[ky * 5 + kx, g] = patch[py::2, px::2].reshape(961)
                    g += 1
    d["col1"] = col.astype(ml_dtypes.bfloat16)

    d["w1t"] = np.asarray(inputs["conv1_w"], np.float32).reshape(16, 25).T.astype(ml_dtypes.bfloat16)
    d["bn1"] = np.stack([np.asarray(inputs[k], np.float32) for k in
                         ("bn1_g", "bn1_b", "bn1_m", "bn1_v", "conv1_b")], axis=1)
    d["bn2"] = np.stack([np.asarray(inputs[k], np.float32) for k in
                         ("bn2_g", "bn2_b", "bn2_m", "bn2_v", "conv2_b")], axis=1)
    # conv2 taps: w2t[tap] = conv2_w[:, :, ky, kx].T  -> [9, 16, 32]
    w2 = np.asarray(inputs["conv2_w"], np.float32)            # [32,16,3,3]
    d["w2t"] = np.transpose(w2, (2, 3, 1, 0)).reshape(9, 16, 32).astype(ml_dtypes.bfloat16)

    # fc0 weights: k-major, k padded to c*256+j, bf16, chunked [64,128,512]
    wf0 = np.asarray(inputs["fc0_w"], np.float32)             # [512, 7200]
    wk = np.zeros((KPAD, 512), np.float32)
    wk.reshape(32, 256, 512)[:, :225, :] = \
        wf0.T.reshape(32, 225, 512)
    d["wf0t"] = wk.astype(ml_dtypes.bfloat16).reshape(NCHUNK, 128, 512)
    d["f0b"] = np.asarray(inputs["fc0_b"], np.float32).reshape(1, 512)

    # fc1: column-producing layout  h_col = fc1_w @ h1 : lhsT chunks [128k,128o]
    wf1 = np.asarray(inputs["fc1_w"], np.float32)             # [128, 512]
    d["wf1t"] = wf1.T.reshape(4, 128, 128).copy()
    d["f1b"] = np.asarray(inputs["fc1_b"], np.float32).reshape(128, 1)

    # state branch
    d["state_c"] = np.asarray(inputs["state"], np.float32).reshape(4, 1)
    d["wf2t"] = np.asarray(inputs["fc2_w"], np.float32).T.copy()   # [4, 64]
    d["f2b"] = np.asarray(inputs["fc2_b"], np.float32).reshape(64, 1)
    d["wf3t"] = np.asarray(inputs["fc3_w"], np.float32).T.copy()   # [64, 64]
    d["f3b"] = np.asarray(inputs["fc3_b"], np.float32).reshape(64, 1)

    # GNN branch: adjacency from edge_index (A[d,s] += 1), transposed
    ei = np.asarray(inputs["edge_index"])
    A = np.zeros((NODE, NODE), np.float32)
    np.add.at(A, (ei[1], ei[0]), 1.0)
    d["at"] = A.T.copy()                                      # [19s, 19d]
    d["xn_c"] = np.asarray(inputs["x_graph"], np.float32).reshape(NODE, 1)
    d["xn_r"] = np.asarray(inputs["x_graph"], np.float32).reshape(1, NODE)
    wl1 = np.asarray(inputs["sage1_wl"], np.float32).reshape(128)
    wr1 = np.asarray(inputs["sage1_wr"], np.float32).reshape(128)
    d["wlr1"] = np.stack([wl1, wr1], axis=0)                  # [2, 128]
    d["bl1"] = np.asarray(inputs["sage1_bl"], np.float32).reshape(128, 1)
    d["wl2t"] = np.asarray(inputs["sage2_wl"], np.float32).T.copy()  # [128,128]
    d["wr2t"] = np.asarray(inputs["sage2_wr"], np.float32).T.copy()
    d["bl2"] = np.asarray(inputs["sage2_bl"], np.float32).reshape(128, 1)
    d["wfgt"] = np.asarray(inputs["fcg_w"], np.float32).T.copy()    # [128, 64]
    d["fgb"] = np.asarray(inputs["fcg_b"], np.float32).reshape(64, 1)

    # head
    wf4 = np.asarray(inputs["fc4_w"], np.float32)             # [128, 256]
    d["wf4t"] = wf4.T.reshape(2, 128, 128).copy()
    d["f4b"] = np.asarray(inputs["fc4_b"], np.float32).reshape(128, 1)
    d["watc"] = np.concatenate(
        [np.asarray(inputs["actor_w"], np.float32).T,
         np.asarray(inputs["critic_w"], np.float32).T], axis=1)  # [128, 20]
    d["abc"] = np.concatenate(
        [np.asarray(inputs["actor_b"], np.float32).reshape(19),
         np.asarray(inputs["critic_b"], np.float32).reshape(1)]).reshape(1, 20)
    return d


# --------------------------------------------------------------------------
# device program (SPMD, identical on all cores)
# --------------------------------------------------------------------------

def build_nc():
    nc = bacc.Bacc(None, target_bir_lowering=False, num_devices=N_CORES)

    def inp(name, shape, dtype=F32):
        return nc.dram_tensor(name, list(shape), dtype, kind="ExternalInput")

    col1 = inp("col1", [25, 4, 961], BF16)
    w1t = inp("w1t", [25, 16], BF16)
    bn1 = inp("bn1", [16, 5])
    bn2 = inp("bn2", [32, 5])
    w2t = inp("w2t", [9, 16, 32], BF16)
    wf0t = inp("wf0t", [NCHUNK, 128, 512], BF16)
    f0b = inp("f0b", [1, 512])
    wf1t = inp("wf1t", [4, 128, 128])
    f1b = inp("f1b", [128, 1])
    state_c = inp("state_c", [4, 1])
    wf2t = inp("wf2t", [4, 64])
    f2b = inp("f2b", [64, 1])
    wf3t = inp("wf3t", [64, 64])
    f3b = inp("f3b", [64, 1])
    at = inp("at", [NODE, NODE])
    xn_c = inp("xn_c", [NODE, 1])
    xn_r = inp("xn_r", [1, NODE])
    wlr1 = inp("wlr1", [2, 128])
    bl1 = inp("bl1", [128, 1])
    wl2t = inp("wl2t", [128, 128])
    wr2t = inp("wr2t", [128, 128])
    bl2 = inp("bl2", [128, 1])
    wfgt = inp("wfgt", [128, 64])
    fgb = inp("fgb", [64, 1])
    wf4t = inp("wf4t", [2, 128, 128])
    f4b = inp("f4b", [128, 1])
    watc = inp("watc", [128, 20])
    abc = inp("abc", [1, 20])

    out = nc.dram_tensor("out", [1, 20], F32, kind="ExternalOutput")

    ident = nc.inline_tensor(np.eye(128, dtype=np.float32), "ident")

    with TileContext(nc) as tc:
        with (
            tc.tile_pool(name="wpool", bufs=1) as wpool,
            tc.tile_pool(name="spool", bufs=1) as spool,
            tc.tile_pool(name="ppool", bufs=2, space="PSUM") as ppool,
            tc.tile_pool(name="cpool", bufs=1, space="PSUM") as cpool,
            tc.tile_pool(name="fpool", bufs=1, space="PSUM") as fpool,
        ):
            # ---------------- early inputs first (conv path) ----------------
            col1_sb = wpool.tile([25, 4, 961], BF16)
            nc.sync.dma_start(col1_sb[:], col1[:])
            w1t_sb = wpool.tile([25, 16], BF16)
            nc.sync.dma_start(w1t_sb[:], w1t[:])
            bn1_sb = wpool.tile([16, 5], F32)
            nc.sync.dma_start(bn1_sb[:], bn1[:])
            bn2_sb = wpool.tile([32, 5], F32)
            nc.sync.dma_start(bn2_sb[:], bn2[:])
            w2t_sb = wpool.tile([16, 9, 32], BF16)
            nc.sync.dma_start(w2t_sb[:], w2t.rearrange("t c o -> c t o"))
            f0b_sb = wpool.tile([1, 512], F32)
            nc.sync.dma_start(f0b_sb[:], f0b[:])
            wf1t_sb = wpool.tile([128, 4, 128], F32)
            nc.sync.dma_start(wf1t_sb[:], wf1t.rearrange("g p f -> p g f"))
            f1b_sb = wpool.tile([128, 1], F32)
            nc.sync.dma_start(f1b_sb[:], f1b[:])
            state_sb = spool.tile([4, 1], F32)
            nc.sync.dma_start(state_sb[:], state_c[:])
            wf2t_sb = spool.tile([4, 64], F32)
            nc.sync.dma_start(wf2t_sb[:], wf2t[:])
            f2b_sb = spool.tile([64, 1], F32)
            nc.sync.dma_start(f2b_sb[:], f2b[:])
            wf3t_sb = spool.tile([64, 64], F32)
            nc.sync.dma_start(wf3t_sb[:], wf3t[:])
            f3b_sb = spool.tile([64, 1], F32)
            nc.sync.dma_start(f3b_sb[:], f3b[:])
            at_sb = spool.tile([NODE, NODE], F32)
            nc.sync.dma_start(at_sb[:], at[:])
            xnc_sb = spool.tile([NODE, 1], F32)
            nc.sync.dma_start(xnc_sb[:], xn_c[:])
            wlr1_sb = spool.tile([2, 128], F32)
            nc.sync.dma_start(wlr1_sb[:], wlr1[:])
            bl1_sb = spool.tile([128, 1], F32)
            nc.sync.dma_start(bl1_sb[:], bl1[:])
            wl2t_sb = spool.tile([128, 128], F32)
            nc.sync.dma_start(wl2t_sb[:], wl2t[:])
            wr2t_sb = spool.tile([128, 128], F32)
            nc.sync.dma_start(wr2t_sb[:], wr2t[:])
            bl2_sb = spool.tile([128, 1], F32)
            nc.sync.dma_start(bl2_sb[:], bl2[:])
            wfgt_sb = spool.tile([128, 64], F32)
            nc.sync.dma_start(wfgt_sb[:], wfgt[:])
            fgb_sb = spool.tile([64, 1], F32)
            nc.sync.dma_start(fgb_sb[:], fgb[:])
            wf4t_sb = spool.tile([128, 2, 128], F32)
            nc.sync.dma_start(wf4t_sb[:], wf4t.rearrange("g p f -> p g f"))
            f4b_sb = spool.tile([128, 1], F32)
            nc.sync.dma_start(f4b_sb[:], f4b[:])
            watc_sb = spool.tile([128, 20], F32)
            nc.sync.dma_start(watc_sb[:], watc[:])
            abc_sb = spool.tile([1, 20], F32)
            nc.sync.dma_start(abc_sb[:], abc[:])
            ident_sb = spool.tile([128, 128], F32)
            nc.sync.dma_start(ident_sb[:], ident[:])

            # ---------------- big fc0 weight stream (8 independent tiles) ----
            wf0_sb = [wpool.tile([128, 8, 512], BF16, name=f"wf0sb{i}")
                      for i in range(8)]
            for i in range(8):
                nc.sync.dma_start(wf0_sb[i][:],
                                  wf0t[i * 8:(i + 1) * 8].rearrange("c p f -> p c f"))

            # preload the Exp activation table off the critical path
            dummy = spool.tile([1, 1], F32)
            nc.vector.memset(dummy[:], 0.0)
            nc.scalar.activation(dummy[:], dummy[:], AF.Exp)


            # ---------------- bn scale/shift from raw params -------------
            # inv = g / sqrt(v + eps);  shift = b - m*inv + conv_b*inv
            def bn_prep(bnp, ch):
                invt = spool.tile([ch, 1], F32, tag=f"bninv{ch}")
                sht = spool.tile([ch, 1], F32, tag=f"bnsh{ch}")
                tmp = spool.tile([ch, 1], F32, tag=f"bntmp{ch}")
                nc.vector.tensor_scalar_add(tmp[:], bnp[:, 3:4], BN_EPS)
                nc.scalar.activation(tmp[:], tmp[:], AF.Sqrt)
                nc.vector.reciprocal(invt[:], tmp[:])
                nc.vector.tensor_tensor(invt[:], invt[:], bnp[:, 0:1], AL.mult)
                # shift = (conv_b - m) * inv + b
                nc.vector.tensor_tensor(sht[:], bnp[:, 4:5], bnp[:, 2:3],
                                        AL.subtract)
                nc.vector.tensor_tensor(sht[:], sht[:], invt[:], AL.mult)
                nc.vector.tensor_tensor(sht[:], sht[:], bnp[:, 1:2], AL.add)
                return invt, sht

            inv1, sh1 = bn_prep(bn1_sb, 16)
            inv2, sh2 = bn_prep(bn2_sb, 32)

            # ---------------- conv1 (4 pool-parity groups) ----------------
            gpsum = [ppool.tile([16, 961], F32, tag="c1g", name=f"c1g{i}") for i in range(4)]
            for g in range(4):
                nc.tensor.matmul(gpsum[g][:, 0:512], w1t_sb[:],
                                 col1_sb[:, g, 0:512], start=True, stop=False)
                nc.tensor.matmul(gpsum[g][:, 512:961], w1t_sb[:],
                                 col1_sb[:, g, 512:961], start=True, stop=True)
            # maxpool = elementwise max of the 4 groups
            gsb = [spool.tile([16, 961], BF16, name=f"gsb{i}") for i in range(4)]
            for i in range(4):
                nc.scalar.activation(gsb[i][:], gpsum[i][:], AF.Copy)
            mx0 = spool.tile([16, 961], BF16)
            mx1 = spool.tile([16, 961], BF16)
            nc.vector.tensor_tensor(mx0[:], gsb[0][:], gsb[1][:], AL.max)
            nc.vector.tensor_tensor(mx1[:], gsb[2][:], gsb[3][:], AL.max)
            nc.vector.tensor_tensor(mx0[:], mx0[:], mx1[:], AL.max)
            # bn1 + relu into 4 x/y-parity planes (conv2 then reads
            # contiguous 2D tiles instead of stride-2 APs)
            pp = spool.tile([16, 4, 16, 16], BF16)
            mx3 = mx0[:].rearrange("c (y x) -> c y x", y=31)
            for py in range(2):
                for px in range(2):
                    ny, nx = (16, 16) if (py, px) == (0, 0) else                         (16 - py, 16 - px)
                    nc.scalar.activation(
                        pp[:, py * 2 + px, 0:ny, 0:nx],
                        mx3[:, py::2, px::2], AF.Relu,
                        bias=sh1[:], scale=inv1[:])

            # ---------------- conv2 (9 tap matmuls, K=16) ----------------
            # pooled viewed as [16, 31, 31]; tap (ky,kx) reads strided 15x15
            c2psum = fpool.tile([32, 225], F32)
            for t in range(9):
                ky, kx = divmod(t, 3)
                plane = (ky % 2) * 2 + (kx % 2)
                rhs = pp[:, plane, ky // 2:ky // 2 + 15,
                         kx // 2:kx // 2 + 15]
                nc.tensor.matmul(c2psum[:], w2t_sb[:, t, :], rhs,
                                 start=(t == 0), stop=(t == 8))
            # bn2 + relu -> h2p [32, 256] bf16 (cols 225:256 zero)
            h2p = spool.tile([32, 256], F32)
            nc.vector.memset(h2p[:, 224:256], 0.0)
            nc.scalar.activation(h2p[:, 0:225], c2psum[:], AF.Relu,
                                 bias=sh2[:], scale=inv2[:])

            # ---------------- h columnization (2 PE transposes) -----------
            # hcol[:, 2t]   = h2p[:, 0:128].T   column t
            # hcol[:, 2t+1] = h2p[:, 128:256].T column t
            hcol = spool.tile([128, 64], BF16)
            tp = cpool.tile([128, 32], F32, tag="med")
            tp2 = cpool.tile([128, 32], F32, tag="med")
            nc.tensor.transpose(tp[:], h2p[:, 0:128], ident_sb[0:32, 0:32])
            nc.tensor.transpose(tp2[:], h2p[:, 128:256], ident_sb[0:32, 0:32])
            nc.scalar.activation(hcol[:].rearrange("p (t two) -> p two t", two=2)
                                 [:, 0, :], tp[:], AF.Copy)
            nc.scalar.activation(hcol[:].rearrange("p (t two) -> p two t", two=2)
                                 [:, 1, :], tp2[:], AF.Copy)

            # ---------------- fc0: 64 accumulating matmuls ----------------
            h1psum = fpool.tile([1, 512], F32)
            for q in range(NCHUNK):
                nc.tensor.matmul(h1psum[:], hcol[:, q:q + 1],
                                 wf0_sb[q // 8][:, q % 8, :], start=(q == 0),
                                 stop=(q == NCHUNK - 1))
            # + bias, relu -> h1 row [1, 512]
            h1row = spool.tile([1, 512], F32)
            nc.vector.tensor_tensor(h1row[:], h1psum[:], f0b_sb[:], AL.add)
            nc.scalar.activation(h1row[:], h1row[:], AF.Relu)

            # columnize h1 (4 transposes of [1,128] -> [128,1])
            h1col = spool.tile([128, 4], F32)
            for g in range(4):
                tpg = cpool.tile([128, 1], F32, tag="small")
                nc.tensor.transpose(tpg[:], h1row[:, g * 128:(g + 1) * 128],
                                    ident_sb[0:1, 0:1])
                nc.scalar.activation(h1col[:, g:g + 1], tpg[:], AF.Copy)

            # ---------------- fc1 -> h_col [128, 1] ----------------
            hpsum = cpool.tile([128, 1], F32, tag="small")
            for g in range(4):
                nc.tensor.matmul(hpsum[:], wf1t_sb[:, g, :], h1col[:, g:g + 1],
                                 start=(g == 0), stop=(g == 3))
            comb0 = spool.tile([128, 1], F32)
            nc.scalar.activation(comb0[:], hpsum[:], AF.Relu, bias=f1b_sb[:])

            # ---------------- state branch -> comb1[0:64] ----------------
            comb1 = spool.tile([128, 1], F32)
            s1ps = cpool.tile([64, 1], F32, tag="small")
            nc.tensor.matmul(s1ps[:], wf2t_sb[:], state_sb[:])
            s1c = spool.tile([64, 1], F32)
            nc.scalar.activation(s1c[:], s1ps[:], AF.Relu, bias=f2b_sb[:])
            s2ps = cpool.tile([64, 1], F32, tag="small")
            nc.tensor.matmul(s2ps[:], wf3t_sb[:], s1c[:])
            nc.scalar.activation(comb1[0:64, :], s2ps[:], AF.Relu,
                                 bias=f3b_sb[:])

            # ---------------- GNN branch -> comb1[64:128] ----------------
            # agg1_row = (A @ xn).T = xn.T @ A.T
            a1ps = cpool.tile([1, NODE], F32, tag="small")
            nc.tensor.matmul(a1ps[:], xnc_sb[:], at_sb[:])
            rhs2 = spool.tile([2, NODE], F32)
            nc.scalar.activation(rhs2[0:1, :], a1ps[:], AF.Copy)
            nc.sync.dma_start(rhs2[1:2, :], xn_r[:])
            # G1T = relu(wlr1.T @ [agg1; xn] + bl1)  [128, 19]
            g1ps = cpool.tile([128, NODE], F32, tag="med")
            nc.tensor.matmul(g1ps[:], wlr1_sb[:], rhs2[:])
            g1t = spool.tile([128, NODE], F32)
            nc.scalar.activation(g1t[:], g1ps[:], AF.Relu, bias=bl1_sb[:])
            # G1 = G1T.T  [19, 128]
            g1tp = cpool.tile([NODE, 128], F32, tag="med")
            nc.tensor.transpose(g1tp[:], g1t[:], ident_sb[:])
            g1 = spool.tile([NODE, 128], F32)
            nc.scalar.activation(g1[:], g1tp[:], AF.Copy)
            # agg2T = G1.T @ A.T  [128, 19]
            a2ps = cpool.tile([128, NODE], F32, tag="med")
            nc.tensor.matmul(a2ps[:], g1[:], at_sb[:])
            a2t = spool.tile([128, NODE], F32)
            nc.scalar.activation(a2t[:], a2ps[:], AF.Copy)
            # G2T = relu(wl2.T.T @ agg2T + wr2.T.T @ G1T + bl2)
            g2ps = cpool.tile([128, NODE], F32, tag="med")
            nc.tensor.matmul(g2ps[:], wl2t_sb[:], a2t[:], start=True, stop=False)
            nc.tensor.matmul(g2ps[:], wr2t_sb[:], g1t[:], start=False, stop=True)
            g2t = spool.tile([128, NODE], F32)
            nc.scalar.activation(g2t[:], g2ps[:], AF.Relu, bias=bl2_sb[:])
            # gsum_col [128,1]; g_col = relu(fcg_w @ gsum / 19 + fgb)
            gsum = spool.tile([128, 1], F32)
            nc.vector.tensor_reduce(gsum[:], g2t[:], mybir.AxisListType.X,
                                    AL.add)
            gps = cpool.tile([64, 1], F32, tag="small")
            nc.tensor.matmul(gps[:], wfgt_sb[:], gsum[:])
            nc.scalar.activation(comb1[64:128, :], gps[:], AF.Relu,
                                 bias=fgb_sb[:], scale=1.0 / 19.0)

            # ---------------- fc4 -> feat_col [128, 1] ----------------
            fps = cpool.tile([128, 1], F32, tag="small")
            nc.tensor.matmul(fps[:], wf4t_sb[:, 0, :], comb0[:],
                             start=True, stop=False)
            nc.tensor.matmul(fps[:], wf4t_sb[:, 1, :], comb1[:],
                             start=False, stop=True)
            feat = spool.tile([128, 1], F32)
            nc.scalar.activation(feat[:], fps[:], AF.Relu, bias=f4b_sb[:])

            # ---------------- actor/critic + softmax ----------------
            zps = cpool.tile([1, 20], F32, tag="small")
            nc.tensor.matmul(zps[:], feat[:], watc_sb[:])
            z = spool.tile([1, 20], F32)
            nc.vector.tensor_tensor(z[:], zps[:], abc_sb[:], AL.add)
            mx = spool.tile([1, 1], F32)
            nc.vector.tensor_reduce(mx[:], z[:, 0:19], mybir.AxisListType.X,
                                    AL.max, negate=True)
            ez = spool.tile([1, 20], F32)
            sexp = spool.tile([1, 1], F32)
            nc.scalar.activation(ez[:, 0:19], z[:, 0:19], AF.Exp,
                                 bias=mx[:], accum_out=sexp[:])
            rs = spool.tile([1, 1], F32)
            nc.vector.reciprocal(rs[:], sexp[:])
            ot = spool.tile([1, 20], F32)
            nc.vector.tensor_scalar(ot[:, 0:19], ez[:, 0:19], rs[:], None,
                                    AL.mult)
            nc.vector.tensor_copy(ot[:, 19:20], z[:, 19:20])
            nc.sync.dma_start(out[:], ot[:])

    nc.compile()
    return nc


_NC_CACHE = None


def kernel(**inputs):
    global _NC_CACHE
    d = _host_prep(inputs)
    if _NC_CACHE is None:
        _NC_CACHE = build_nc()
    nc = _NC_CACHE
    in_maps = [dict(d) for _ in range(N_CORES)]
    r = run_bass_kernel_spmd(nc, in_maps, core_ids=list(range(N_CORES)))
    o = np.asarray(r.results[0]["out"], np.float32).reshape(20)
    probs = o[:19].reshape(1, 19).astype(np.float32)
    value = o[19:].reshape(1, 1).astype(np.float32)
    return probs, value


# revision 16
# speedup vs baseline: 1.0395x; 1.0395x over previous
"""Trainium2 Bass kernel for nn_ActorCritic (CNN + MLP + 19-node GNN, batch 1).

Strategy: the model is tiny except fc0_w (512x7200 f32 = 14.7MB), and the
network is a single serial chain ending in 20 output scalars, so there is no
way to split it across cores without a cross-core combine -- and on this
runtime every cross-core primitive (collective_compute, remote DMA) costs
~85us in entry-barrier/firmware latency, dwarfing the 5us saved on DMA.
So each of the 8 cores runs the full network independently (identical
outputs; core 0's is returned).  The dominant fc0_w stream is cast to
bfloat16 on the host (rel-err ~3e-3, well inside tolerance), halving the
memory-bound phase, and is consumed by the TensorEngine as 512-wide moving
operands while the conv chain overlaps the stream.
"""

import numpy as np
import ml_dtypes

import concourse.bacc as bacc
import concourse.mybir as mybir
from concourse.tile import TileContext
from concourse.bass_utils import run_bass_kernel_spmd

N_CORES = 8
F32 = mybir.dt.float32
BF16 = mybir.dt.bfloat16
AL = mybir.AluOpType
AF = mybir.ActivationFunctionType

NODE = 19
BN_EPS = 1e-5

# fc0 k-dim padded layout: k' = c*256 + j   (c<32 conv2-channels, j<225 pixels)
KPAD = 32 * 256          # 8192
NCHUNK = KPAD // 128     # 64


# --------------------------------------------------------------------------
# host-side input prep (pure relayouts / gathers, no model arithmetic)
# --------------------------------------------------------------------------

def _host_prep(inputs):
    d = {}
    x = np.asarray(inputs["x"], np.float32).reshape(125, 125)

    # conv1 im2col, grouped by maxpool 2x2 output parity.
    # conv1: 5x5 stride 2 pad 1 -> 62x62; pool 2x2 -> 31x31 per parity group.
    xp = np.zeros((128, 128), np.float32)
    xp[1:126, 1:126] = x  # zero pad=1 (plus dead rows/cols beyond)
    col = np.empty((25, 4, 961), np.float32)
    for ky in range(5):
        for kx in range(5):
            # conv out (y,x): input (2y+ky, 2x+kx) in padded coords
            patch = xp[ky:ky + 124:2, kx:kx + 124:2]          # [62, 62]
            g = 0
            for py in range(2):
                for px in range(2):
                    col[ky * 5 + kx, g] = patch[py::2, px::2].reshape(961)
                    g += 1
    d["col1"] = col.astype(ml_dtypes.bfloat16)

    d["w1t"] = np.asarray(inputs["conv1_w"], np.float32).reshape(16, 25).T.astype(ml_dtypes.bfloat16)
    d["bn1"] = np.stack([np.asarray(inputs[k], np.float32) for k in
                         ("bn1_g", "bn1_b", "bn1_m", "bn1_v", "conv1_b")], axis=1)
    d["bn2"] = np.stack([np.asarray(inputs[k], np.float32) for k in
                         ("bn2_g", "bn2_b", "bn2_m", "bn2_v", "conv2_b")], axis=1)
    # conv2 taps: w2t[tap] = conv2_w[:, :, ky, kx].T  -> [9, 16, 32]
    w2 = np.asarray(inputs["conv2_w"], np.float32)            # [32,16,3,3]
    d["w2t"] = np.transpose(w2, (2, 3, 1, 0)).reshape(9, 16, 32).astype(ml_dtypes.bfloat16)

    # fc0 weights: k-major, k padded to c*256+j, bf16, chunked [64,128,512]
    wf0 = np.asarray(inputs["fc0_w"], np.float32)             # [512, 7200]
    wk = np.zeros((KPAD, 512), np.float32)
    wk.reshape(32, 256, 512)[:, :225, :] = \
        wf0.T.reshape(32, 225, 512)
    wkb = wk.astype(ml_dtypes.bfloat16).reshape(8, 8, 128, 512)
    d["wf0t"] = np.ascontiguousarray(np.transpose(wkb, (0, 2, 1, 3))
                                     ).reshape(8, 128, 4096)
    d["f0b"] = np.asarray(inputs["fc0_b"], np.float32).reshape(1, 512)

    # fc1: column-producing layout  h_col = fc1_w @ h1 : lhsT chunks [128k,128o]
    wf1 = np.asarray(inputs["fc1_w"], np.float32)             # [128, 512]
    d["wf1t"] = wf1.T.reshape(4, 128, 128).copy()
    d["f1b"] = np.asarray(inputs["fc1_b"], np.float32).reshape(128, 1)

    # state branch
    d["state_c"] = np.asarray(inputs["state"], np.float32).reshape(4, 1)
    d["wf2t"] = np.asarray(inputs["fc2_w"], np.float32).T.copy()   # [4, 64]
    d["f2b"] = np.asarray(inputs["fc2_b"], np.float32).reshape(64, 1)
    d["wf3t"] = np.asarray(inputs["fc3_w"], np.float32).T.copy()   # [64, 64]
    d["f3b"] = np.asarray(inputs["fc3_b"], np.float32).reshape(64, 1)

    # GNN branch: adjacency from edge_index (A[d,s] += 1), transposed
    ei = np.asarray(inputs["edge_index"])
    A = np.zeros((NODE, NODE), np.float32)
    np.add.at(A, (ei[1], ei[0]), 1.0)
    d["at"] = A.T.copy()                                      # [19s, 19d]
    d["xn_c"] = np.asarray(inputs["x_graph"], np.float32).reshape(NODE, 1)
    d["xn_r"] = np.asarray(inputs["x_graph"], np.float32).reshape(1, NODE)
    wl1 = np.asarray(inputs["sage1_wl"], np.float32).reshape(128)
    wr1 = np.asarray(inputs["sage1_wr"], np.float32).reshape(128)
    d["wlr1"] = np.stack([wl1, wr1], axis=0)                  # [2, 128]
    d["bl1"] = np.asarray(inputs["sage1_bl"], np.float32).reshape(128, 1)
    d["wl2t"] = np.asarray(inputs["sage2_wl"], np.float32).T.copy()  # [128,128]
    d["wr2t"] = np.asarray(inputs["sage2_wr"], np.float32).T.copy()
    d["bl2"] = np.asarray(inputs["sage2_bl"], np.float32).reshape(128, 1)
    d["wfgt"] = np.asarray(inputs["fcg_w"], np.float32).T.copy()    # [128, 64]
    d["fgb"] = np.asarray(inputs["fcg_b"], np.float32).reshape(64, 1)

    # head
    wf4 = np.asarray(inputs["fc4_w"], np.float32)             # [128, 256]
    d["wf4t"] = wf4.T.reshape(2, 128, 128).copy()
    d["f4b"] = np.asarray(inputs["fc4_b"], np.float32).reshape(128, 1)
    d["watc"] = np.concatenate(
        [np.asarray(inputs["actor_w"], np.float32).T,
         np.asarray(inputs["critic_w"], np.float32).T], axis=1)  # [128, 20]
    d["abc"] = np.concatenate(
        [np.asarray(inputs["actor_b"], np.float32).reshape(19),
         np.asarray(inputs["critic_b"], np.float32).reshape(1)]).reshape(1, 20)
    return d


# --------------------------------------------------------------------------
# device program (SPMD, identical on all cores)
# --------------------------------------------------------------------------

def build_nc():
    nc = bacc.Bacc(None, target_bir_lowering=False, num_devices=N_CORES)

    def inp(name, shape, dtype=F32):
        return nc.dram_tensor(name, list(shape), dtype, kind="ExternalInput")

    col1 = inp("col1", [25, 4, 961], BF16)
    w1t = inp("w1t", [25, 16], BF16)
    bn1 = inp("bn1", [16, 5])
    bn2 = inp("bn2", [32, 5])
    w2t = inp("w2t", [9, 16, 32], BF16)
    wf0t = inp("wf0t", [8, 128, 4096], BF16)
    f0b = inp("f0b", [1, 512])
    wf1t = inp("wf1t", [4, 128, 128])
    f1b = inp("f1b", [128, 1])
    state_c = inp("state_c", [4, 1])
    wf2t = inp("wf2t", [4, 64])
    f2b = inp("f2b", [64, 1])
    wf3t = inp("wf3t", [64, 64])
    f3b = inp("f3b", [64, 1])
    at = inp("at", [NODE, NODE])
    xn_c = inp("xn_c", [NODE, 1])
    xn_r = inp("xn_r", [1, NODE])
    wlr1 = inp("wlr1", [2, 128])
    bl1 = inp("bl1", [128, 1])
    wl2t = inp("wl2t", [128, 128])
    wr2t = inp("wr2t", [128, 128])
    bl2 = inp("bl2", [128, 1])
    wfgt = inp("wfgt", [128, 64])
    fgb = inp("fgb", [64, 1])
    wf4t = inp("wf4t", [2, 128, 128])
    f4b = inp("f4b", [128, 1])
    watc = inp("watc", [128, 20])
    abc = inp("abc", [1, 20])

    out = nc.dram_tensor("out", [1, 20], F32, kind="ExternalOutput")

    ident = nc.inline_tensor(np.eye(128, dtype=np.float32), "ident")

    with TileContext(nc) as tc:
        with (
            tc.tile_pool(name="wpool", bufs=1) as wpool,
            tc.tile_pool(name="spool", bufs=1) as spool,
            tc.tile_pool(name="ppool", bufs=2, space="PSUM") as ppool,
            tc.tile_pool(name="cpool", bufs=1, space="PSUM") as cpool,
            tc.tile_pool(name="fpool", bufs=1, space="PSUM") as fpool,
        ):
            # ---------------- early inputs first (conv path) ----------------
            col1_sb = wpool.tile([25, 4, 961], BF16)
            nc.sync.dma_start(col1_sb[:], col1[:])
            w1t_sb = wpool.tile([25, 16], BF16)
            nc.sync.dma_start(w1t_sb[:], w1t[:])
            bn1_sb = wpool.tile([16, 5], F32)
            nc.sync.dma_start(bn1_sb[:], bn1[:])
            bn2_sb = wpool.tile([32, 5], F32)
            nc.sync.dma_start(bn2_sb[:], bn2[:])
            w2t_sb = wpool.tile([16, 9, 32], BF16)
            nc.sync.dma_start(w2t_sb[:], w2t.rearrange("t c o -> c t o"))
            f0b_sb = wpool.tile([1, 512], F32)
            nc.sync.dma_start(f0b_sb[:], f0b[:])
            wf1t_sb = wpool.tile([128, 4, 128], F32)
            nc.sync.dma_start(wf1t_sb[:], wf1t.rearrange("g p f -> p g f"))
            f1b_sb = wpool.tile([128, 1], F32)
            nc.sync.dma_start(f1b_sb[:], f1b[:])
            state_sb = spool.tile([4, 1], F32)
            nc.sync.dma_start(state_sb[:], state_c[:])
            wf2t_sb = spool.tile([4, 64], F32)
            nc.sync.dma_start(wf2t_sb[:], wf2t[:])
            f2b_sb = spool.tile([64, 1], F32)
            nc.sync.dma_start(f2b_sb[:], f2b[:])
            wf3t_sb = spool.tile([64, 64], F32)
            nc.sync.dma_start(wf3t_sb[:], wf3t[:])
            f3b_sb = spool.tile([64, 1], F32)
            nc.sync.dma_start(f3b_sb[:], f3b[:])
            at_sb = spool.tile([NODE, NODE], F32)
            nc.sync.dma_start(at_sb[:], at[:])
            xnc_sb = spool.tile([NODE, 1], F32)
            nc.sync.dma_start(xnc_sb[:], xn_c[:])
            wlr1_sb = spool.tile([2, 128], F32)
            nc.sync.dma_start(wlr1_sb[:], wlr1[:])
            bl1_sb = spool.tile([128, 1], F32)
            nc.sync.dma_start(bl1_sb[:], bl1[:])
            wl2t_sb = spool.tile([128, 128], F32)
            nc.sync.dma_start(wl2t_sb[:], wl2t[:])
            wr2t_sb = spool.tile([128, 128], F32)
            nc.sync.dma_start(wr2t_sb[:], wr2t[:])
            bl2_sb = spool.tile([128, 1], F32)
            nc.sync.dma_start(bl2_sb[:], bl2[:])
            wfgt_sb = spool.tile([128, 64], F32)
            nc.sync.dma_start(wfgt_sb[:], wfgt[:])
            fgb_sb = spool.tile([64, 1], F32)
            nc.sync.dma_start(fgb_sb[:], fgb[:])
            wf4t_sb = spool.tile([128, 2, 128], F32)
            nc.sync.dma_start(wf4t_sb[:], wf4t.rearrange("g p f -> p g f"))
            f4b_sb = spool.tile([128, 1], F32)
            nc.sync.dma_start(f4b_sb[:], f4b[:])
            watc_sb = spool.tile([128, 20], F32)
            nc.sync.dma_start(watc_sb[:], watc[:])
            abc_sb = spool.tile([1, 20], F32)
            nc.sync.dma_start(abc_sb[:], abc[:])
            ident_sb = spool.tile([128, 128], F32)
            nc.sync.dma_start(ident_sb[:], ident[:])

            # ---------------- big fc0 weight stream (8 independent tiles) ----
            wf0_sb = [wpool.tile([128, 8, 512], BF16, name=f"wf0sb{i}")
                      for i in range(8)]
            for i in range(8):
                nc.sync.dma_start(
                    wf0_sb[i][:].rearrange("p c f -> p (c f)"), wf0t[i])

            # preload the Exp activation table off the critical path
            dummy = spool.tile([1, 1], F32)
            nc.vector.memset(dummy[:], 0.0)
            nc.scalar.activation(dummy[:], dummy[:], AF.Exp)


            # ---------------- bn scale/shift from raw params -------------
            # inv = g / sqrt(v + eps);  shift = b - m*inv + conv_b*inv
            def bn_prep(bnp, ch):
                invt = spool.tile([ch, 1], F32, tag=f"bninv{ch}")
                sht = spool.tile([ch, 1], F32, tag=f"bnsh{ch}")
                tmp = spool.tile([ch, 1], F32, tag=f"bntmp{ch}")
                nc.vector.tensor_scalar_add(tmp[:], bnp[:, 3:4], BN_EPS)
                nc.scalar.activation(tmp[:], tmp[:], AF.Sqrt)
                nc.vector.reciprocal(invt[:], tmp[:])
                nc.vector.tensor_tensor(invt[:], invt[:], bnp[:, 0:1], AL.mult)
                # shift = (conv_b - m) * inv + b
                nc.vector.tensor_tensor(sht[:], bnp[:, 4:5], bnp[:, 2:3],
                                        AL.subtract)
                nc.vector.tensor_tensor(sht[:], sht[:], invt[:], AL.mult)
                nc.vector.tensor_tensor(sht[:], sht[:], bnp[:, 1:2], AL.add)
                return invt, sht

            inv1, sh1 = bn_prep(bn1_sb, 16)
            inv2, sh2 = bn_prep(bn2_sb, 32)

            # ---------------- state branch -> comb1[0:64] ----------------
            comb1 = spool.tile([128, 1], F32)
            s1ps = cpool.tile([64, 1], F32, tag="small")
            nc.tensor.matmul(s1ps[:], wf2t_sb[:], state_sb[:])
            s1c = spool.tile([64, 1], F32)
            nc.scalar.activation(s1c[:], s1ps[:], AF.Relu, bias=f2b_sb[:])
            s2ps = cpool.tile([64, 1], F32, tag="small")
            nc.tensor.matmul(s2ps[:], wf3t_sb[:], s1c[:])
            nc.scalar.activation(comb1[0:64, :], s2ps[:], AF.Relu,
                                 bias=f3b_sb[:])

            # ---------------- GNN branch -> comb1[64:128] ----------------
            # agg1_row = (A @ xn).T = xn.T @ A.T
            a1ps = cpool.tile([1, NODE], F32, tag="small")
            nc.tensor.matmul(a1ps[:], xnc_sb[:], at_sb[:])
            rhs2 = spool.tile([2, NODE], F32)
            nc.scalar.activation(rhs2[0:1, :], a1ps[:], AF.Copy)
            nc.sync.dma_start(rhs2[1:2, :], xn_r[:])
            # G1T = relu(wlr1.T @ [agg1; xn] + bl1)  [128, 19]
            g1ps = cpool.tile([128, NODE], F32, tag="med", bufs=2)
            nc.tensor.matmul(g1ps[:], wlr1_sb[:], rhs2[:])
            g1t = spool.tile([128, NODE], F32)
            nc.scalar.activation(g1t[:], g1ps[:], AF.Relu, bias=bl1_sb[:])
            # G1 = G1T.T  [19, 128]
            g1tp = cpool.tile([NODE, 128], F32, tag="med", bufs=2)
            nc.tensor.transpose(g1tp[:], g1t[:], ident_sb[:])
            g1 = spool.tile([NODE, 128], F32)
            nc.scalar.activation(g1[:], g1tp[:], AF.Copy)
            # agg2T = G1.T @ A.T  [128, 19]
            a2ps = cpool.tile([128, NODE], F32, tag="med", bufs=2)
            nc.tensor.matmul(a2ps[:], g1[:], at_sb[:])
            a2t = spool.tile([128, NODE], F32)
            nc.scalar.activation(a2t[:], a2ps[:], AF.Copy)
            # G2T = relu(wl2.T.T @ agg2T + wr2.T.T @ G1T + bl2)
            g2ps = cpool.tile([128, NODE], F32, tag="med", bufs=2)
            nc.tensor.matmul(g2ps[:], wl2t_sb[:], a2t[:], start=True, stop=False)
            nc.tensor.matmul(g2ps[:], wr2t_sb[:], g1t[:], start=False, stop=True)
            g2t = spool.tile([128, NODE], F32)
            nc.scalar.activation(g2t[:], g2ps[:], AF.Relu, bias=bl2_sb[:])
            # gsum_col [128,1]; g_col = relu(fcg_w @ gsum / 19 + fgb)
            gsum = spool.tile([128, 1], F32)
            nc.vector.tensor_reduce(gsum[:], g2t[:], mybir.AxisListType.X,
                                    AL.add)
            gps = cpool.tile([64, 1], F32, tag="small")
            nc.tensor.matmul(gps[:], wfgt_sb[:], gsum[:])
            nc.scalar.activation(comb1[64:128, :], gps[:], AF.Relu,
                                 bias=fgb_sb[:], scale=1.0 / 19.0)

            # ---------------- conv1 (4 pool-parity groups) ----------------
            gpsum = [ppool.tile([16, 961], F32, tag="c1g", name=f"c1g{i}") for i in range(4)]
            for g in range(4):
                nc.tensor.matmul(gpsum[g][:, 0:512], w1t_sb[:],
                                 col1_sb[:, g, 0:512], start=True, stop=False)
                nc.tensor.matmul(gpsum[g][:, 512:961], w1t_sb[:],
                                 col1_sb[:, g, 512:961], start=True, stop=True)
            # maxpool = elementwise max of the 4 groups
            gsb = [spool.tile([16, 961], BF16, name=f"gsb{i}") for i in range(4)]
            for i in range(4):
                nc.scalar.activation(gsb[i][:], gpsum[i][:], AF.Copy)
            mx0 = spool.tile([16, 961], BF16)
            mx1 = spool.tile([16, 961], BF16)
            nc.vector.tensor_tensor(mx0[:], gsb[0][:], gsb[1][:], AL.max)
            nc.vector.tensor_tensor(mx1[:], gsb[2][:], gsb[3][:], AL.max)
            nc.vector.tensor_tensor(mx0[:], mx0[:], mx1[:], AL.max)
            # bn1 + relu into 4 x/y-parity planes (conv2 then reads
            # contiguous 2D tiles instead of stride-2 APs)
            pp = spool.tile([16, 4, 16, 16], BF16)
            mx3 = mx0[:].rearrange("c (y x) -> c y x", y=31)
            for py in range(2):
                for px in range(2):
                    ny, nx = (16, 16) if (py, px) == (0, 0) else                         (16 - py, 16 - px)
                    nc.scalar.activation(
                        pp[:, py * 2 + px, 0:ny, 0:nx],
                        mx3[:, py::2, px::2], AF.Relu,
                        bias=sh1[:], scale=inv1[:])

            # ---------------- conv2 (9 tap matmuls, K=16) ----------------
            # pooled viewed as [16, 31, 31]; tap (ky,kx) reads strided 15x15
            c2psum = fpool.tile([32, 225], F32, tag="acc")
            for t in range(9):
                ky, kx = divmod(t, 3)
                plane = (ky % 2) * 2 + (kx % 2)
                rhs = pp[:, plane, ky // 2:ky // 2 + 15,
                         kx // 2:kx // 2 + 15]
                nc.tensor.matmul(c2psum[:], w2t_sb[:, t, :], rhs,
                                 start=(t == 0), stop=(t == 8))
            # bn2 + relu -> h2p [32, 256] bf16 (cols 225:256 zero)
            h2p = spool.tile([32, 256], F32)
            nc.vector.memset(h2p[:, 224:256], 0.0)
            nc.scalar.activation(h2p[:, 0:225], c2psum[:], AF.Relu,
                                 bias=sh2[:], scale=inv2[:])

            # ---------------- h columnization (2 PE transposes) -----------
            # hcol[:, 2t]   = h2p[:, 0:128].T   column t
            # hcol[:, 2t+1] = h2p[:, 128:256].T column t
            hcol = spool.tile([128, 64], BF16)
            tp = cpool.tile([128, 32], F32, tag="med", bufs=2)
            tp2 = cpool.tile([128, 32], F32, tag="med", bufs=2)
            nc.tensor.transpose(tp[:], h2p[:, 0:128], ident_sb[0:32, 0:32])
            nc.tensor.transpose(tp2[:], h2p[:, 128:256], ident_sb[0:32, 0:32])
            nc.scalar.activation(hcol[:].rearrange("p (t two) -> p two t", two=2)
                                 [:, 0, :], tp[:], AF.Copy)
            nc.scalar.activation(hcol[:].rearrange("p (t two) -> p two t", two=2)
                                 [:, 1, :], tp2[:], AF.Copy)

            # ---------------- fc0: 64 accumulating matmuls ----------------
            h1psum = fpool.tile([1, 512], F32, tag="acc")
            for q in range(NCHUNK):
                nc.tensor.matmul(h1psum[:], hcol[:, q:q + 1],
                                 wf0_sb[q // 8][:, q % 8, :], start=(q == 0),
                                 stop=(q == NCHUNK - 1))
            # + bias, relu -> h1 row [1, 512]
            h1row = spool.tile([1, 512], F32)
            nc.vector.tensor_tensor(h1row[:], h1psum[:], f0b_sb[:], AL.add)
            nc.scalar.activation(h1row[:], h1row[:], AF.Relu)

            # columnize h1 (4 transposes of [1,128] -> [128,1])
            h1col = spool.tile([128, 4], F32)
            for g in range(4):
                tpg = cpool.tile([128, 1], F32, tag="small")
                nc.tensor.transpose(tpg[:], h1row[:, g * 128:(g + 1) * 128],
                                    ident_sb[0:1, 0:1])
                nc.scalar.activation(h1col[:, g:g + 1], tpg[:], AF.Copy)

            # ---------------- fc1 -> h_col [128, 1] ----------------
            hpsum = cpool.tile([128, 1], F32, tag="small")
            for g in range(4):
                nc.tensor.matmul(hpsum[:], wf1t_sb[:, g, :], h1col[:, g:g + 1],
                                 start=(g == 0), stop=(g == 3))
            comb0 = spool.tile([128, 1], F32)
            nc.scalar.activation(comb0[:], hpsum[:], AF.Relu, bias=f1b_sb[:])

            # ---------------- fc4 -> feat_col [128, 1] ----------------
            fps = cpool.tile([128, 1], F32, tag="small")
            nc.tensor.matmul(fps[:], wf4t_sb[:, 0, :], comb0[:],
                             start=True, stop=False)
            nc.tensor.matmul(fps[:], wf4t_sb[:, 1, :], comb1[:],
                             start=False, stop=True)
            feat = spool.tile([128, 1], F32)
            nc.scalar.activation(feat[:], fps[:], AF.Relu, bias=f4b_sb[:])

            # ---------------- actor/critic + softmax ----------------
            zps = cpool.tile([1, 20], F32, tag="small")
            nc.tensor.matmul(zps[:], feat[:], watc_sb[:])
            z = spool.tile([1, 20], F32)
            nc.vector.tensor_tensor(z[:], zps[:], abc_sb[:], AL.add)
            mx = spool.tile([1, 1], F32)
            nc.vector.tensor_reduce(mx[:], z[:, 0:19], mybir.AxisListType.X,
                                    AL.max, negate=True)
            ez = spool.tile([1, 20], F32)
            sexp = spool.tile([1, 1], F32)
            nc.scalar.activation(ez[:, 0:19], z[:, 0:19], AF.Exp,
                                 bias=mx[:], accum_out=sexp[:])
            rs = spool.tile([1, 1], F32)
            nc.vector.reciprocal(rs[:], sexp[:])
            ot = spool.tile([1, 20], F32)
            nc.vector.tensor_scalar(ot[:, 0:19], ez[:, 0:19], rs[:], None,
                                    AL.mult)
            nc.vector.tensor_copy(ot[:, 19:20], z[:, 19:20])
            nc.sync.dma_start(out[:], ot[:])

    nc.compile()
    return nc


_NC_CACHE = None


def kernel(**inputs):
    global _NC_CACHE
    d = _host_prep(inputs)
    if _NC_CACHE is None:
        _NC_CACHE = build_nc()
    nc = _NC_CACHE
    in_maps = [dict(d) for _ in range(N_CORES)]
    r = run_bass_kernel_spmd(nc, in_maps, core_ids=list(range(N_CORES)))
    o = np.asarray(r.results[0]["out"], np.float32).reshape(20)
    probs = o[:19].reshape(1, 19).astype(np.float32)
    value = o[19:].reshape(1, 1).astype(np.float32)
    return probs, value


# revision 17
# speedup vs baseline: 1.3037x; 1.2542x over previous
"""Trainium2 Bass kernel for nn_ActorCritic (CNN + MLP + 19-node GNN, batch 1).

Strategy: the model is tiny except fc0_w (512x7200 f32 = 14.7MB), and the
network is a single serial chain ending in 20 output scalars, so there is no
way to split it across cores without a cross-core combine -- and on this
runtime every cross-core primitive (collective_compute, remote DMA) costs
~85us in entry-barrier/firmware latency, dwarfing the 5us saved on DMA.
So each of the 8 cores runs the full network independently (identical
outputs; core 0's is returned).  The dominant fc0_w stream is cast to
bfloat16 on the host (rel-err ~3e-3, well inside tolerance), halving the
memory-bound phase, and is consumed by the TensorEngine as 512-wide moving
operands while the conv chain overlaps the stream.
"""

import numpy as np
import ml_dtypes

import concourse.bacc as bacc
import concourse.mybir as mybir
from concourse.tile import TileContext
from concourse.bass_utils import run_bass_kernel_spmd

N_CORES = 8
F32 = mybir.dt.float32
BF16 = mybir.dt.bfloat16
AL = mybir.AluOpType
AF = mybir.ActivationFunctionType

NODE = 19
BN_EPS = 1e-5

# fc0 k-dim padded layout: k' = c*256 + j   (c<32 conv2-channels, j<225 pixels)
KPAD = 32 * 256          # 8192
NCHUNK = KPAD // 128     # 64


# --------------------------------------------------------------------------
# host-side input prep (pure relayouts / gathers, no model arithmetic)
# --------------------------------------------------------------------------

def _host_prep(inputs):
    d = {}
    x = np.asarray(inputs["x"], np.float32).reshape(125, 125)

    # conv1 im2col, grouped by maxpool 2x2 output parity.
    # conv1: 5x5 stride 2 pad 1 -> 62x62; pool 2x2 -> 31x31 per parity group.
    xp = np.zeros((128, 128), np.float32)
    xp[1:126, 1:126] = x  # zero pad=1 (plus dead rows/cols beyond)
    col = np.empty((25, 4, 961), np.float32)
    for ky in range(5):
        for kx in range(5):
            # conv out (y,x): input (2y+ky, 2x+kx) in padded coords
            patch = xp[ky:ky + 124:2, kx:kx + 124:2]          # [62, 62]
            g = 0
            for py in range(2):
                for px in range(2):
                    col[ky * 5 + kx, g] = patch[py::2, px::2].reshape(961)
                    g += 1
    d["col1"] = col.astype(ml_dtypes.bfloat16)

    d["w1t"] = np.asarray(inputs["conv1_w"], np.float32).reshape(16, 25).T.astype(ml_dtypes.bfloat16)
    d["bn1"] = np.stack([np.asarray(inputs[k], np.float32) for k in
                         ("bn1_g", "bn1_b", "bn1_m", "bn1_v", "conv1_b")], axis=1)
    d["bn2"] = np.stack([np.asarray(inputs[k], np.float32) for k in
                         ("bn2_g", "bn2_b", "bn2_m", "bn2_v", "conv2_b")], axis=1)
    # conv2 taps: w2t[tap] = conv2_w[:, :, ky, kx].T  -> [9, 16, 32]
    w2 = np.asarray(inputs["conv2_w"], np.float32)            # [32,16,3,3]
    d["w2t"] = np.transpose(w2, (2, 3, 1, 0)).reshape(9, 16, 32).astype(ml_dtypes.bfloat16)

    # fc0 weights: k-major, k padded to c*256+j, bf16, chunked [64,128,512]
    wf0 = np.asarray(inputs["fc0_w"], np.float32)             # [512, 7200]
    wk = np.zeros((KPAD, 512), np.float32)
    wk.reshape(32, 256, 512)[:, :225, :] = \
        wf0.T.reshape(32, 225, 512)
    wkb = wk.astype(ml_dtypes.bfloat16).reshape(8, 8, 128, 512)
    d["wf0t"] = np.ascontiguousarray(np.transpose(wkb, (0, 2, 1, 3))
                                     ).reshape(8, 128, 4096)
    d["f0b"] = np.asarray(inputs["fc0_b"], np.float32).reshape(1, 512)

    # fc1: column-producing layout  h_col = fc1_w @ h1 : lhsT chunks [128k,128o]
    wf1 = np.asarray(inputs["fc1_w"], np.float32)             # [128, 512]
    d["wf1t"] = wf1.T.reshape(4, 128, 128).copy()
    d["f1b"] = np.asarray(inputs["fc1_b"], np.float32).reshape(128, 1)

    # state branch
    d["state_c"] = np.asarray(inputs["state"], np.float32).reshape(4, 1)
    d["wf2t"] = np.asarray(inputs["fc2_w"], np.float32).T.copy()   # [4, 64]
    d["f2b"] = np.asarray(inputs["fc2_b"], np.float32).reshape(64, 1)
    d["wf3t"] = np.asarray(inputs["fc3_w"], np.float32).T.copy()   # [64, 64]
    d["f3b"] = np.asarray(inputs["fc3_b"], np.float32).reshape(64, 1)

    # GNN branch: adjacency from edge_index (A[d,s] += 1), transposed
    ei = np.asarray(inputs["edge_index"])
    A = np.zeros((NODE, NODE), np.float32)
    np.add.at(A, (ei[1], ei[0]), 1.0)
    d["at"] = A.T.copy()                                      # [19s, 19d]
    d["xn_c"] = np.asarray(inputs["x_graph"], np.float32).reshape(NODE, 1)
    d["xn_r"] = np.asarray(inputs["x_graph"], np.float32).reshape(1, NODE)
    wl1 = np.asarray(inputs["sage1_wl"], np.float32).reshape(128)
    wr1 = np.asarray(inputs["sage1_wr"], np.float32).reshape(128)
    d["wlr1"] = np.stack([wl1, wr1], axis=0)                  # [2, 128]
    d["bl1"] = np.asarray(inputs["sage1_bl"], np.float32).reshape(128, 1)
    d["wl2t"] = np.asarray(inputs["sage2_wl"], np.float32).T.copy()  # [128,128]
    d["wr2t"] = np.asarray(inputs["sage2_wr"], np.float32).T.copy()
    d["bl2"] = np.asarray(inputs["sage2_bl"], np.float32).reshape(128, 1)
    d["wfgt"] = np.asarray(inputs["fcg_w"], np.float32).T.copy()    # [128, 64]
    d["fgb"] = np.asarray(inputs["fcg_b"], np.float32).reshape(64, 1)

    # head
    wf4 = np.asarray(inputs["fc4_w"], np.float32)             # [128, 256]
    d["wf4t"] = wf4.T.reshape(2, 128, 128).copy()
    d["f4b"] = np.asarray(inputs["fc4_b"], np.float32).reshape(128, 1)
    d["watc"] = np.concatenate(
        [np.asarray(inputs["actor_w"], np.float32).T,
         np.asarray(inputs["critic_w"], np.float32).T], axis=1)  # [128, 20]
    d["abc"] = np.concatenate(
        [np.asarray(inputs["actor_b"], np.float32).reshape(19),
         np.asarray(inputs["critic_b"], np.float32).reshape(1)]).reshape(1, 20)
    return d


# --------------------------------------------------------------------------
# device program (SPMD, identical on all cores)
# --------------------------------------------------------------------------

def build_nc():
    nc = bacc.Bacc(None, target_bir_lowering=False, num_devices=N_CORES)

    def inp(name, shape, dtype=F32):
        return nc.dram_tensor(name, list(shape), dtype, kind="ExternalInput")

    col1 = inp("col1", [25, 4, 961], BF16)
    w1t = inp("w1t", [25, 16], BF16)
    bn1 = inp("bn1", [16, 5])
    bn2 = inp("bn2", [32, 5])
    w2t = inp("w2t", [9, 16, 32], BF16)
    wf0t = inp("wf0t", [8, 128, 4096], BF16)
    f0b = inp("f0b", [1, 512])
    wf1t = inp("wf1t", [4, 128, 128])
    f1b = inp("f1b", [128, 1])
    state_c = inp("state_c", [4, 1])
    wf2t = inp("wf2t", [4, 64])
    f2b = inp("f2b", [64, 1])
    wf3t = inp("wf3t", [64, 64])
    f3b = inp("f3b", [64, 1])
    at = inp("at", [NODE, NODE])
    xn_c = inp("xn_c", [NODE, 1])
    xn_r = inp("xn_r", [1, NODE])
    wlr1 = inp("wlr1", [2, 128])
    bl1 = inp("bl1", [128, 1])
    wl2t = inp("wl2t", [128, 128])
    wr2t = inp("wr2t", [128, 128])
    bl2 = inp("bl2", [128, 1])
    wfgt = inp("wfgt", [128, 64])
    fgb = inp("fgb", [64, 1])
    wf4t = inp("wf4t", [2, 128, 128])
    f4b = inp("f4b", [128, 1])
    watc = inp("watc", [128, 20])
    abc = inp("abc", [1, 20])

    out = nc.dram_tensor("out", [1, 20], F32, kind="ExternalOutput")

    ident = nc.inline_tensor(np.eye(128, dtype=np.float32), "ident")

    with TileContext(nc) as tc:
        with (
            tc.tile_pool(name="wpool", bufs=1) as wpool,
            tc.tile_pool(name="spool", bufs=1) as spool,
            tc.tile_pool(name="ppool", bufs=2, space="PSUM") as ppool,
            tc.tile_pool(name="cpool", bufs=1, space="PSUM") as cpool,
            tc.tile_pool(name="fpool", bufs=1, space="PSUM") as fpool,
        ):
            # ---------------- early inputs first (conv path) ----------------
            col1_sb = wpool.tile([25, 4, 961], BF16)
            nc.sync.dma_start(col1_sb[:], col1[:])
            w1t_sb = wpool.tile([25, 16], BF16)
            nc.sync.dma_start(w1t_sb[:], w1t[:])
            bn1_sb = wpool.tile([16, 5], F32)
            nc.sync.dma_start(bn1_sb[:], bn1[:])
            bn2_sb = wpool.tile([32, 5], F32)
            nc.sync.dma_start(bn2_sb[:], bn2[:])
            w2t_sb = wpool.tile([16, 9, 32], BF16)
            nc.sync.dma_start(w2t_sb[:], w2t.rearrange("t c o -> c t o"))
            f0b_sb = wpool.tile([1, 512], F32)
            nc.sync.dma_start(f0b_sb[:], f0b[:])
            wf1t_sb = wpool.tile([128, 4, 128], F32)
            nc.sync.dma_start(wf1t_sb[:], wf1t.rearrange("g p f -> p g f"))
            f1b_sb = wpool.tile([128, 1], F32)
            nc.sync.dma_start(f1b_sb[:], f1b[:])
            state_sb = spool.tile([4, 1], F32)
            nc.sync.dma_start(state_sb[:], state_c[:])
            wf2t_sb = spool.tile([4, 64], F32)
            nc.sync.dma_start(wf2t_sb[:], wf2t[:])
            f2b_sb = spool.tile([64, 1], F32)
            nc.sync.dma_start(f2b_sb[:], f2b[:])
            wf3t_sb = spool.tile([64, 64], F32)
            nc.sync.dma_start(wf3t_sb[:], wf3t[:])
            f3b_sb = spool.tile([64, 1], F32)
            nc.sync.dma_start(f3b_sb[:], f3b[:])
            at_sb = spool.tile([NODE, NODE], F32)
            nc.sync.dma_start(at_sb[:], at[:])
            xnc_sb = spool.tile([NODE, 1], F32)
            nc.sync.dma_start(xnc_sb[:], xn_c[:])
            wlr1_sb = spool.tile([2, 128], F32)
            nc.sync.dma_start(wlr1_sb[:], wlr1[:])
            bl1_sb = spool.tile([128, 1], F32)
            nc.sync.dma_start(bl1_sb[:], bl1[:])
            wl2t_sb = spool.tile([128, 128], F32)
            nc.sync.dma_start(wl2t_sb[:], wl2t[:])
            wr2t_sb = spool.tile([128, 128], F32)
            nc.sync.dma_start(wr2t_sb[:], wr2t[:])
            bl2_sb = spool.tile([128, 1], F32)
            nc.sync.dma_start(bl2_sb[:], bl2[:])
            wfgt_sb = spool.tile([128, 64], F32)
            nc.sync.dma_start(wfgt_sb[:], wfgt[:])
            fgb_sb = spool.tile([64, 1], F32)
            nc.sync.dma_start(fgb_sb[:], fgb[:])
            wf4t_sb = spool.tile([128, 2, 128], F32)
            nc.sync.dma_start(wf4t_sb[:], wf4t.rearrange("g p f -> p g f"))
            f4b_sb = spool.tile([128, 1], F32)
            nc.sync.dma_start(f4b_sb[:], f4b[:])
            watc_sb = spool.tile([128, 20], F32)
            nc.sync.dma_start(watc_sb[:], watc[:])
            abc_sb = spool.tile([1, 20], F32)
            nc.sync.dma_start(abc_sb[:], abc[:])
            ident_sb = spool.tile([128, 128], F32)
            nc.sync.dma_start(ident_sb[:], ident[:])
            rhs2 = spool.tile([2, NODE], F32)
            nc.sync.dma_start(rhs2[1:2, :], xn_r[:])

            # ---------------- big fc0 weight stream (8 independent tiles) ----
            wf0_sb = [wpool.tile([128, 8, 512], BF16, name=f"wf0sb{i}")
                      for i in range(8)]
            for i in range(8):
                nc.sync.dma_start(
                    wf0_sb[i][:].rearrange("p c f -> p (c f)"), wf0t[i])

            # preload the Exp activation table off the critical path
            dummy = spool.tile([1, 1], F32)
            nc.vector.memset(dummy[:], 0.0)
            nc.scalar.activation(dummy[:], dummy[:], AF.Exp)


            # ---------------- bn scale/shift from raw params -------------
            # inv = g / sqrt(v + eps);  shift = b - m*inv + conv_b*inv
            def bn_prep(bnp, ch):
                invt = spool.tile([ch, 1], F32, tag=f"bninv{ch}")
                sht = spool.tile([ch, 1], F32, tag=f"bnsh{ch}")
                tmp = spool.tile([ch, 1], F32, tag=f"bntmp{ch}")
                nc.vector.tensor_scalar_add(tmp[:], bnp[:, 3:4], BN_EPS)
                nc.scalar.activation(tmp[:], tmp[:], AF.Sqrt)
                nc.vector.reciprocal(invt[:], tmp[:])
                nc.vector.tensor_tensor(invt[:], invt[:], bnp[:, 0:1], AL.mult)
                # shift = (conv_b - m) * inv + b
                nc.vector.tensor_tensor(sht[:], bnp[:, 4:5], bnp[:, 2:3],
                                        AL.subtract)
                nc.vector.tensor_tensor(sht[:], sht[:], invt[:], AL.mult)
                nc.vector.tensor_tensor(sht[:], sht[:], bnp[:, 1:2], AL.add)
                return invt, sht

            inv1, sh1 = bn_prep(bn1_sb, 16)
            inv2, sh2 = bn_prep(bn2_sb, 32)

            # ---------------- state branch -> comb1[0:64] ----------------
            comb1 = spool.tile([128, 1], F32)
            s1ps = cpool.tile([64, 1], F32, tag="small")
            nc.tensor.matmul(s1ps[:], wf2t_sb[:], state_sb[:])
            s1c = spool.tile([64, 1], F32)
            nc.scalar.activation(s1c[:], s1ps[:], AF.Relu, bias=f2b_sb[:])
            s2ps = cpool.tile([64, 1], F32, tag="small")
            nc.tensor.matmul(s2ps[:], wf3t_sb[:], s1c[:])
            nc.scalar.activation(comb1[0:64, :], s2ps[:], AF.Relu,
                                 bias=f3b_sb[:])

            # ---------------- GNN branch -> comb1[64:128] ----------------
            # agg1_row = (A @ xn).T = xn.T @ A.T
            a1ps = cpool.tile([1, NODE], F32, tag="small")
            nc.tensor.matmul(a1ps[:], xnc_sb[:], at_sb[:])
            nc.scalar.activation(rhs2[0:1, :], a1ps[:], AF.Copy)
            # G1T = relu(wlr1.T @ [agg1; xn] + bl1)  [128, 19]
            g1ps = cpool.tile([128, NODE], F32, tag="med", bufs=2)
            nc.tensor.matmul(g1ps[:], wlr1_sb[:], rhs2[:])
            g1t = spool.tile([128, NODE], F32)
            nc.scalar.activation(g1t[:], g1ps[:], AF.Relu, bias=bl1_sb[:])
            # G1 = G1T.T  [19, 128]
            g1tp = cpool.tile([NODE, 128], F32, tag="med", bufs=2)
            nc.tensor.transpose(g1tp[:], g1t[:], ident_sb[:])
            g1 = spool.tile([NODE, 128], F32)
            nc.scalar.activation(g1[:], g1tp[:], AF.Copy)
            # agg2T = G1.T @ A.T  [128, 19]
            a2ps = cpool.tile([128, NODE], F32, tag="med", bufs=2)
            nc.tensor.matmul(a2ps[:], g1[:], at_sb[:])
            a2t = spool.tile([128, NODE], F32)
            nc.scalar.activation(a2t[:], a2ps[:], AF.Copy)
            # G2T = relu(wl2.T.T @ agg2T + wr2.T.T @ G1T + bl2)
            g2ps = cpool.tile([128, NODE], F32, tag="med", bufs=2)
            nc.tensor.matmul(g2ps[:], wl2t_sb[:], a2t[:], start=True, stop=False)
            nc.tensor.matmul(g2ps[:], wr2t_sb[:], g1t[:], start=False, stop=True)
            g2t = spool.tile([128, NODE], F32)
            nc.scalar.activation(g2t[:], g2ps[:], AF.Relu, bias=bl2_sb[:])
            # gsum_col [128,1]; g_col = relu(fcg_w @ gsum / 19 + fgb)
            gsum = spool.tile([128, 1], F32)
            nc.vector.tensor_reduce(gsum[:], g2t[:], mybir.AxisListType.X,
                                    AL.add)
            gps = cpool.tile([64, 1], F32, tag="small")
            nc.tensor.matmul(gps[:], wfgt_sb[:], gsum[:])
            nc.scalar.activation(comb1[64:128, :], gps[:], AF.Relu,
                                 bias=fgb_sb[:], scale=1.0 / 19.0)

            # ---------------- conv1 (4 pool-parity groups) ----------------
            gpsum = [ppool.tile([16, 961], F32, tag="c1g", name=f"c1g{i}") for i in range(4)]
            for g in range(4):
                nc.tensor.matmul(gpsum[g][:, 0:512], w1t_sb[:],
                                 col1_sb[:, g, 0:512], start=True, stop=False)
                nc.tensor.matmul(gpsum[g][:, 512:961], w1t_sb[:],
                                 col1_sb[:, g, 512:961], start=True, stop=True)
            # maxpool = elementwise max of the 4 groups
            gsb = [spool.tile([16, 961], BF16, name=f"gsb{i}") for i in range(4)]
            for i in range(4):
                nc.scalar.activation(gsb[i][:], gpsum[i][:], AF.Copy)
            mx0 = spool.tile([16, 961], BF16)
            mx1 = spool.tile([16, 961], BF16)
            nc.vector.tensor_tensor(mx0[:], gsb[0][:], gsb[1][:], AL.max)
            nc.vector.tensor_tensor(mx1[:], gsb[2][:], gsb[3][:], AL.max)
            nc.vector.tensor_tensor(mx0[:], mx0[:], mx1[:], AL.max)
            # bn1 + relu into 4 x/y-parity planes (conv2 then reads
            # contiguous 2D tiles instead of stride-2 APs)
            pp = spool.tile([16, 4, 16, 16], BF16)
            mx3 = mx0[:].rearrange("c (y x) -> c y x", y=31)
            for py in range(2):
                for px in range(2):
                    ny, nx = (16, 16) if (py, px) == (0, 0) else                         (16 - py, 16 - px)
                    nc.scalar.activation(
                        pp[:, py * 2 + px, 0:ny, 0:nx],
                        mx3[:, py::2, px::2], AF.Relu,
                        bias=sh1[:], scale=inv1[:])

            # ---------------- conv2 (9 tap matmuls, K=16) ----------------
            # pooled viewed as [16, 31, 31]; tap (ky,kx) reads strided 15x15
            c2psum = fpool.tile([32, 225], F32, tag="acc")
            for t in range(9):
                ky, kx = divmod(t, 3)
                plane = (ky % 2) * 2 + (kx % 2)
                rhs = pp[:, plane, ky // 2:ky // 2 + 15,
                         kx // 2:kx // 2 + 15]
                nc.tensor.matmul(c2psum[:], w2t_sb[:, t, :], rhs,
                                 start=(t == 0), stop=(t == 8))
            # bn2 + relu -> h2p [32, 256] bf16 (cols 225:256 zero)
            h2p = spool.tile([32, 256], F32)
            nc.vector.memset(h2p[:, 224:256], 0.0)
            nc.scalar.activation(h2p[:, 0:225], c2psum[:], AF.Relu,
                                 bias=sh2[:], scale=inv2[:])

            # ---------------- h columnization (2 PE transposes) -----------
            # hcol[:, 2t]   = h2p[:, 0:128].T   column t
            # hcol[:, 2t+1] = h2p[:, 128:256].T column t
            hcol = spool.tile([128, 64], BF16)
            tp = cpool.tile([128, 32], F32, tag="med", bufs=2)
            tp2 = cpool.tile([128, 32], F32, tag="med", bufs=2)
            nc.tensor.transpose(tp[:], h2p[:, 0:128], ident_sb[0:32, 0:32])
            nc.tensor.transpose(tp2[:], h2p[:, 128:256], ident_sb[0:32, 0:32])
            nc.scalar.activation(hcol[:].rearrange("p (t two) -> p two t", two=2)
                                 [:, 0, :], tp[:], AF.Copy)
            nc.scalar.activation(hcol[:].rearrange("p (t two) -> p two t", two=2)
                                 [:, 1, :], tp2[:], AF.Copy)

            # ---------------- fc0: 64 accumulating matmuls ----------------
            h1psum = fpool.tile([1, 512], F32, tag="acc")
            for q in range(NCHUNK):
                nc.tensor.matmul(h1psum[:], hcol[:, q:q + 1],
                                 wf0_sb[q // 8][:, q % 8, :], start=(q == 0),
                                 stop=(q == NCHUNK - 1))
            # + bias, relu -> h1 row [1, 512]
            h1row = spool.tile([1, 512], F32)
            nc.vector.tensor_tensor(h1row[:], h1psum[:], f0b_sb[:], AL.add)
            nc.scalar.activation(h1row[:], h1row[:], AF.Relu)

            # columnize h1 (4 transposes of [1,128] -> [128,1])
            h1col = spool.tile([128, 4], F32)
            for g in range(4):
                tpg = cpool.tile([128, 1], F32, tag="small")
                nc.tensor.transpose(tpg[:], h1row[:, g * 128:(g + 1) * 128],
                                    ident_sb[0:1, 0:1])
                nc.scalar.activation(h1col[:, g:g + 1], tpg[:], AF.Copy)

            # ---------------- fc1 -> h_col [128, 1] ----------------
            hpsum = cpool.tile([128, 1], F32, tag="small")
            for g in range(4):
                nc.tensor.matmul(hpsum[:], wf1t_sb[:, g, :], h1col[:, g:g + 1],
                                 start=(g == 0), stop=(g == 3))
            comb0 = spool.tile([128, 1], F32)
            nc.scalar.activation(comb0[:], hpsum[:], AF.Relu, bias=f1b_sb[:])

            # ---------------- fc4 -> feat_col [128, 1] ----------------
            fps = cpool.tile([128, 1], F32, tag="small")
            nc.tensor.matmul(fps[:], wf4t_sb[:, 0, :], comb0[:],
                             start=True, stop=False)
            nc.tensor.matmul(fps[:], wf4t_sb[:, 1, :], comb1[:],
                             start=False, stop=True)
            feat = spool.tile([128, 1], F32)
            nc.scalar.activation(feat[:], fps[:], AF.Relu, bias=f4b_sb[:])

            # ---------------- actor/critic + softmax ----------------
            zps = cpool.tile([1, 20], F32, tag="small")
            nc.tensor.matmul(zps[:], feat[:], watc_sb[:])
            z = spool.tile([1, 20], F32)
            nc.vector.tensor_tensor(z[:], zps[:], abc_sb[:], AL.add)
            mx = spool.tile([1, 1], F32)
            nc.vector.tensor_reduce(mx[:], z[:, 0:19], mybir.AxisListType.X,
                                    AL.max, negate=True)
            ez = spool.tile([1, 20], F32)
            sexp = spool.tile([1, 1], F32)
            nc.scalar.activation(ez[:, 0:19], z[:, 0:19], AF.Exp,
                                 bias=mx[:], accum_out=sexp[:])
            rs = spool.tile([1, 1], F32)
            nc.vector.reciprocal(rs[:], sexp[:])
            ot = spool.tile([1, 20], F32)
            nc.vector.tensor_scalar(ot[:, 0:19], ez[:, 0:19], rs[:], None,
                                    AL.mult)
            nc.vector.tensor_copy(ot[:, 19:20], z[:, 19:20])
            nc.sync.dma_start(out[:], ot[:])

    nc.compile()
    return nc


_NC_CACHE = None


def kernel(**inputs):
    global _NC_CACHE
    d = _host_prep(inputs)
    if _NC_CACHE is None:
        _NC_CACHE = build_nc()
    nc = _NC_CACHE
    in_maps = [dict(d) for _ in range(N_CORES)]
    r = run_bass_kernel_spmd(nc, in_maps, core_ids=list(range(N_CORES)))
    o = np.asarray(r.results[0]["out"], np.float32).reshape(20)
    probs = o[:19].reshape(1, 19).astype(np.float32)
    value = o[19:].reshape(1, 1).astype(np.float32)
    return probs, value
